# revision 14
# baseline (speedup 1.0000x reference)
"""Multi-head attention kernel for Trainium2, data-parallel over batch on 8 cores.

Problem: B=16, N=1024, DIM=768, H=12 heads, head_dim=64, fp32.
  q = x@Wq+bq; k = x@Wk+bk; v = x@Wv+bv   (per-head split)
  out = softmax(q k^T / sqrt(DIM)) v      (per head), concat, @Wo + bo

Sharding: batch-parallel. Each core gets 2 batches and all weights; no
collectives. Output gathered by concat.

Per-core layout strategy (per batch of 1024 tokens):
  - XT = x^T  [768 feat, 1024 tok] via PE transposes (fp32 DMA transpose
    unsupported).
  - QT/KT = (x@W + b)^T [768, 1024]: matmul(lhsT=W, rhs=XT). Head h lives on
    partition rows (h%2)*64..: pair p = m-tile p.
  - V natural [1024 tok, 768] via matmul(lhsT=XT, rhs=Wv), stored per-pair
    padded: [Vh0(64) | ones(1) | pad(31) | Vh1(64)] = 160 cols. The shared
    ones column makes PV emit softmax denominators at 32-aligned psum rows:
      h0: lhsT cols [0:128]  -> psum rows 0-63 = O_h0^T, row 64 = denom_h0
      h1: lhsT cols [32:160] -> psum row 32 = denom_h1, rows 64-127 = O_h1^T
         (remaining rows garbage, never read)
  - S^T[key, q] = matmul(lhsT=KT head rows, rhs=QT head rows), contraction 64,
    two heads row-packed in the PE array (partitions 0-63 / 64-127).
  - P^T = exp(SCALE * S^T) on ACT (no max subtraction needed: |SCALE*S| < ~2),
    [128, 1024] ops (2 key-blocks per op) to amortize ACT overhead.
  - O^T normalized by broadcast reciprocal rows, written to OT [768, 1024].
  - Y = matmul(lhsT=OT, rhs=Wo) + bo -> natural [tok, 768], DMA out.

All matmuls (projections and attention) run in bf16 with fp32 psum
accumulation: the PE moving-operand port is 2 B/lane/cycle on TRN2, so bf16
streams 1 col/cycle while f32r needs 2 — bf16 projections measure ~65us
faster end-to-end than f32r (434us vs 500us same-session), at rel err
~3.8e-3 of the output absmax (budget 2e-2). x is cast to bf16 in DRAM and
transposed via the XBAR DMA-transpose, so the PE does no transpose work.
"""

import sys
import types

sys.path.insert(0, "/opt/trn_rl_repo")

import numpy as np

# Register the axon NTFF profile hook if the image's antenv lacks it (needed
# only when run with trace=True; harmless otherwise).
import antenv  # noqa: F401

if "antenv.axon_hooks" not in sys.modules:
    _hooks_mod = types.ModuleType("antenv.axon_hooks")
    _hooks_mod._hook = None

    def _set_hook(h):
        _hooks_mod._hook = h

    def _get_hook():
        return _hooks_mod._hook

    _hooks_mod.set_axon_ntff_profile_hook = _set_hook
    _hooks_mod.get_axon_ntff_profile_hook = _get_hook
    sys.modules["antenv.axon_hooks"] = _hooks_mod
    try:
        from trn_agent_boot.trn_boot import _ntff_profile_via_ctypes

        _set_hook(_ntff_profile_via_ctypes("/opt/axon/libaxon_pjrt.so"))
    except Exception:
        pass

import concourse.bass_utils as bass_utils

bass_utils.upload_artifacts = lambda tmpdir: f"local:{tmpdir}"  # no bucket creds

import concourse.bacc as bacc
import concourse.mybir as mybir
import concourse.tile as tile
from concourse.bass_utils import run_bass_kernel_spmd
from concourse.masks import make_identity

P = 128
DIM = 768
N_HEADS = 12
HD = 64
N = 1024
B = 16
NCORES = 8
BL = B // NCORES  # batches per core = 2
SCALE = 1.0 / float(np.sqrt(DIM))

KT = DIM // P      # 6 k-tiles of the 768 contraction
TT = N // P        # 8 token tiles per batch
NPAIR = N_HEADS // 2  # 6 head pairs
QC = 512           # query chunk (psum bank, fp32)
PAIRW = 160        # pair block in V_ext: [Vh0(64)|ones(1)|pad(31)|Vh1(64)]

F32 = mybir.dt.float32

_cache = {}


def build(mm_dtype, attn_bf16=True, dbg=False, warm=False):
    nc = bacc.Bacc("TRN2", target_bir_lowering=False, debug=False)

    x = nc.dram_tensor("inputs", [BL, N, DIM], F32, kind="ExternalInput")
    wq = nc.dram_tensor("Wq", [DIM, DIM], F32, kind="ExternalInput")
    bq = nc.dram_tensor("bq", [DIM], F32, kind="ExternalInput")
    wk = nc.dram_tensor("Wk", [DIM, DIM], F32, kind="ExternalInput")
    bk = nc.dram_tensor("bk", [DIM], F32, kind="ExternalInput")
    wv = nc.dram_tensor("Wv", [DIM, DIM], F32, kind="ExternalInput")
    bv = nc.dram_tensor("bv", [DIM], F32, kind="ExternalInput")
    wo = nc.dram_tensor("Wo", [DIM, DIM], F32, kind="ExternalInput")
    bo = nc.dram_tensor("bo", [DIM], F32, kind="ExternalInput")
    out = nc.dram_tensor("out", [BL, N, DIM], F32, kind="ExternalOutput")
    if dbg:
        d_xt = nc.dram_tensor("d_xt", [P, KT, N], F32, kind="ExternalOutput")
        d_vext = nc.dram_tensor("d_vext", [P, TT, NPAIR * PAIRW], F32, kind="ExternalOutput")
        d_qt = nc.dram_tensor("d_qt", [P, N], F32, kind="ExternalOutput")
        d_kt = nc.dram_tensor("d_kt", [P, N], F32, kind="ExternalOutput")
        d_pt0 = nc.dram_tensor("d_pt0", [P, 2 * QC], F32, kind="ExternalOutput")
        d_pt1 = nc.dram_tensor("d_pt1", [P, 2 * QC], F32, kind="ExternalOutput")
        d_osba = nc.dram_tensor("d_osba", [P, QC], F32, kind="ExternalOutput")
        d_osbb = nc.dram_tensor("d_osbb", [P, QC], F32, kind="ExternalOutput")
        d_rb = nc.dram_tensor("d_rb", [P, QC], F32, kind="ExternalOutput")
        d_ot = nc.dram_tensor("d_ot", [P, KT, N], F32, kind="ExternalOutput")

    wq_r = wq.rearrange("(ko ki) m -> ki ko m", ki=P)
    wk_r = wk.rearrange("(ko ki) m -> ki ko m", ki=P)
    wv_r = wv.rearrange("(ko ki) m -> ki ko m", ki=P)
    wo_r = wo.rearrange("(ko ki) m -> ki ko m", ki=P)
    bq_r = bq.rearrange("(ko ki) -> ki ko", ki=P)
    bk_r = bk.rearrange("(ko ki) -> ki ko", ki=P)

    # weights DMA: gpsimd can cast f32 -> f32r/bf16 in flight
    wdma = nc.sync.dma_start if mm_dtype == F32 else nc.gpsimd.dma_start
    attn_dt = mybir.dt.bfloat16 if attn_bf16 else mm_dtype
    xbufs = 3 if mm_dtype == mybir.dt.bfloat16 else 2

    with tile.TileContext(nc) as tc:
        with (
            tc.tile_pool(name="const", bufs=1) as cpool,
            tc.tile_pool(name="work", bufs=1) as pool,
            tc.tile_pool(name="dram", bufs=1, space="DRAM") as dpool,
            tc.tile_pool(name="ps", bufs=1, space="PSUM") as ps,
        ):
            ident = cpool.tile([P, P], F32)
            make_identity(nc, ident)

            if warm:
                # dummy matmuls to hold the DVFS p-state up while the x cast
                # + XBAR transpose + weight-load prologue runs (PE otherwise
                # idles at kernel start and the clock halves). The pipelined
                # per-token-block transposes get real work going ~12us in, so
                # 32 warm matmuls suffice to bridge.
                wrm = cpool.tile([P, QC], mybir.dt.bfloat16)
                nc.vector.memset(wrm[:], 0.0)
                for _ in range(16):
                    wps = ps.tile([P, QC], F32, tag="mm", bufs=2, name="wps")
                    nc.tensor.matmul(
                        wps[:], wrm[:, 0:P], wrm[:], start=True, stop=True
                    )

            bf16_x = mm_dtype == mybir.dt.bfloat16

            # resident weights (full); order by first use. With x loaded on
            # the sync queue (below), the gpsimd cast queue carries ONLY the
            # weights, so wv lands ~9us in and wq/wk by ~23us.
            wq_sb = cpool.tile([P, KT, DIM], mm_dtype)
            wk_sb = cpool.tile([P, KT, DIM], mm_dtype)
            wv_sb = cpool.tile([P, KT, DIM], mm_dtype)
            wo_sb = cpool.tile([P, KT, DIM], mm_dtype)
            for k in range(KT):
                wdma(wv_sb[:, k], wv_r[:, k])
            for k in range(KT):
                wdma(wq_sb[:, k], wq_r[:, k])
                wdma(wk_sb[:, k], wk_r[:, k])
            for k in range(KT):
                wdma(wo_sb[:, k], wo_r[:, k])

            bq_sb = cpool.tile([P, KT], F32)
            bk_sb = cpool.tile([P, KT], F32)
            bv_b = cpool.tile([P, DIM], F32)
            bo_b = cpool.tile([P, DIM], F32)

            # V_ext: [tok_inner, tok_outer, pair blocks of PAIRW cols]
            # cols p*PAIRW + [0:64] = V head 2p, +64 = ones, +[96:160] = V 2p+1
            # pad cols stay uninitialized: they only produce garbage psum rows
            # that are never read. Ones col via DVE cast-copy (f32r producer).
            v_ext = cpool.tile([P, TT, NPAIR * PAIRW], attn_dt)
            ones_src = cpool.tile([P, TT * NPAIR], F32)
            nc.vector.memset(ones_src[:], 1.0)
            ones_cols = v_ext[:].rearrange("p t (np w) -> p t np w", w=PAIRW)[
                :, :, :, 64:65
            ]
            nc.vector.tensor_copy(
                ones_cols,
                ones_src[:].rearrange("p (t np) -> p t np", np=NPAIR)[:, :, :, None],
            )

            # XT via PE transposes: x token blocks stream in as plain f32 on
            # the otherwise-idle sync queue (no cast round-trip through DRAM,
            # no slow small-block XBAR transposes); the PE transposes each
            # 128x128 chunk (f32 transpose = 2 cycles/row, ~107ns/chunk, and
            # doubles as DVFS warm-up work) and the DVE copies psum -> xt in
            # bf16. Both batches run here: b1's transposes fill the wv-paced
            # bubbles of b0's V projection, and issuing its x loads late
            # would park them behind b0's attention-epilogue DMAs (FIFO).
            xts = []
            if bf16_x:
                for b in range(BL):
                    xt_b = pool.tile(
                        [P, KT, N], mm_dtype, tag="xt_ot", bufs=xbufs, name=f"xt{b}"
                    )
                    xts.append(xt_b)
                for b in range(BL):
                    for to in range(TT):
                        tsl = slice(to * P, (to + 1) * P)
                        xstage = pool.tile([P, DIM], F32, tag="xstage", bufs=3)
                        nc.sync.dma_start(xstage[:], x[b, tsl, :])
                        tp0 = ps.tile([P, QC], F32, tag="mm", bufs=2, name="tp0")
                        tp1 = ps.tile([P, QC], F32, tag="mm", bufs=2, name="tp1")
                        for fo in range(KT):
                            dst = tp0 if fo < 4 else tp1
                            nc.tensor.transpose(
                                dst[:, (fo % 4) * P : (fo % 4 + 1) * P],
                                xstage[:, fo * P : (fo + 1) * P],
                                ident,
                            )
                        nc.vector.tensor_copy(
                            xts[b][:, 0:4, tsl],
                            tp0[:].rearrange("p (f t) -> p f t", f=4),
                        )
                        nc.vector.tensor_copy(
                            xts[b][:, 4:6, tsl],
                            tp1[:, 0:256].rearrange("p (f t) -> p f t", f=2),
                        )

            for b in range(BL):
                # ---- XT = x[b]^T ------------------------------------------
                if b == 0:
                    nc.scalar.dma_start(bv_b[:], bv[None, :].to_broadcast((P, DIM)))
                    nc.scalar.dma_start(bq_sb[:], bq_r)
                    nc.scalar.dma_start(bk_sb[:], bk_r)
                    nc.scalar.dma_start(bo_b[:], bo[None, :].to_broadcast((P, DIM)))
                if bf16_x:
                    xt = xts[b]
                else:
                    xt = pool.tile(
                        [P, KT, N], mm_dtype, tag="xt_ot", bufs=xbufs, name="xt"
                    )
                    for to in range(TT):
                        xstage = pool.tile([P, DIM], F32, tag="xstage", bufs=2)
                        nc.sync.dma_start(xstage[:], x[b, to * P : (to + 1) * P, :])
                        for fo in range(KT):
                            tps = ps.tile([P, QC], F32, tag="mm", bufs=2, name="tps")
                            nc.tensor.transpose(
                                tps[:, :P], xstage[:, fo * P : (fo + 1) * P], ident
                            )
                            nc.vector.tensor_copy(
                                xt[:, fo, to * P : (to + 1) * P], tps[:, :P]
                            )

                # ---- V natural + ones layout ------------------------------
                for to in range(TT):
                    vpss = {
                        ch: ps.tile([P, QC], F32, tag="mm", bufs=2, name=f"vps{ch}")
                        for ch in (0, 1)
                    }
                    for k in range(KT):
                        for ch, cw in ((0, 512), (1, 256)):
                            nc.tensor.matmul(
                                vpss[ch][:, :cw],
                                xt[:, k, to * P : (to + 1) * P],
                                wv_sb[:, k, ch * 512 : ch * 512 + cw],
                                start=(k == 0),
                                stop=(k == KT - 1),
                            )
                    for ch, cw in ((0, 512), (1, 256)):
                        vps = vpss[ch]
                        # scatter heads into pair-padded blocks (+bias)
                        npr = cw // (2 * HD)  # pairs in this chunk (4 then 2)
                        pr0 = ch * 4          # first pair in this chunk
                        for par in (0, 1):    # even/odd head of each pair
                            src = vps[:, :cw].rearrange(
                                "p (np two w) -> p np two w", two=2, w=HD
                            )[:, :, par, :]
                            bsrc = bv_b[:, ch * 512 : ch * 512 + cw].rearrange(
                                "p (np two w) -> p np two w", two=2, w=HD
                            )[:, :, par, :]
                            off = 96 if par else 0
                            dst = v_ext[:, to, :].rearrange(
                                "p (np w) -> p np w", w=PAIRW
                            )[:, pr0 : pr0 + npr, off : off + HD]
                            nc.vector.scalar_tensor_tensor(
                                out=dst,
                                in0=src,
                                scalar=1.0,
                                in1=bsrc,
                                op0=mybir.AluOpType.mult,
                                op1=mybir.AluOpType.add,
                            )

                if dbg and b == 0:
                    nc.sync.dma_start(d_xt[:], xt[:].bitcast(F32))
                    nc.sync.dma_start(d_vext[:], v_ext[:].bitcast(F32))

                # ---- OT buffer for this batch -----------------------------
                ot = pool.tile([P, KT, N], mm_dtype, tag="xt_ot", bufs=xbufs, name="ot")

                # ---- per head-pair: QT/KT proj then attention -------------
                def make_qk_work(npo, nqt, nkt):
                    """QK projection for pair npo as a list of closures (one
                    k-step of one dst each; the last also adds the biases) so
                    it can interleave into the PREVIOUS pair's attention — the
                    PE queue is in-order, so the exp-wait stalls there would
                    otherwise leave the PE idle while this work sits queued
                    behind them."""
                    work = []
                    for dst_t, w_t, bias in ((nqt, wq_sb, bq_sb), (nkt, wk_sb, bk_sb)):
                        holder = {}

                        def chunk(k, dst_t=dst_t, w_t=w_t, bias=bias, holder=holder):
                            if k == 0:
                                holder["pp"] = [
                                    ps.tile([P, QC], F32, tag="mm", bufs=2, name=f"pps{qs}")
                                    for qs in range(N // QC)
                                ]
                            for qs in range(N // QC):
                                nc.tensor.matmul(
                                    holder["pp"][qs][:],
                                    w_t[:, k, npo * P : (npo + 1) * P],
                                    xt[:, k, qs * QC : (qs + 1) * QC],
                                    start=(k == 0),
                                    stop=(k == KT - 1),
                                )
                            if k == KT - 1:
                                for qs in range(N // QC):
                                    nc.vector.tensor_scalar_add(
                                        dst_t[:, qs * QC : (qs + 1) * QC],
                                        holder["pp"][qs][:],
                                        bias[:, npo : npo + 1],
                                    )

                        for k in range(KT):
                            work.append(lambda k=k, chunk=chunk: chunk(k))
                    return work

                # pair 0's QK projection runs inline; later pairs' are
                # interleaved into the preceding pair's attention loop
                qt_t = pool.tile([P, N], attn_dt, tag="qt", bufs=xbufs)
                kt_t = pool.tile([P, N], attn_dt, tag="kt", bufs=xbufs)
                for w0 in make_qk_work(0, qt_t, kt_t):
                    w0()

                for po in range(NPAIR):
                    qk_work = []
                    if po + 1 < NPAIR:
                        nqt = pool.tile([P, N], attn_dt, tag="qt", bufs=xbufs)
                        nkt = pool.tile([P, N], attn_dt, tag="kt", bufs=xbufs)
                        qk_work = make_qk_work(po + 1, nqt, nkt)
                        next_tiles = (nqt, nkt)

                    if dbg and b == 0 and po == 0:
                        nc.sync.dma_start(d_qt[:], qt_t[:].bitcast(F32))
                        nc.sync.dma_start(d_kt[:], kt_t[:].bitcast(F32))

                    pb = po * PAIRW
                    for qc in range(N // QC):
                        qsl = slice(qc * QC, (qc + 1) * QC)
                        oa = ps.tile([P, QC], F32, tag="oa", bufs=1, name="oa")
                        ob = ps.tile([P, QC], F32, tag="ob", bufs=1, name="ob")
                        for kb in range(TT):
                            ksl = slice(kb * P, (kb + 1) * P)
                            # ONE st tile per key block holding BOTH heads
                            # ([h0 512 | h1 512]) so a single exp frees both
                            # banks at once: the two score matmuls then become
                            # ready together and issue back-to-back, streaming
                            # CONCURRENTLY on the two 64-row PE tile halves
                            # (h0 rows 0-63, h1 rows 64-127) — 2 cols/cycle.
                            # With separate st0/st1 + two exps, h1's bank
                            # frees ~1.1us after h0's and the scheduler runs
                            # the h0 matmuls solo, breaking the pairing.
                            st = ps.tile([P, 2 * QC], F32, tag="st", bufs=2, name="st")
                            # high_priority keeps the pair adjacent in the
                            # final schedule: filler k-steps become ready
                            # one-at-a-time (chained psum accumulation) and
                            # would otherwise slip between h0 and h64 with
                            # their lower (earlier) priorities.
                            with tc.high_priority(offset=256):
                                nc.tensor.matmul(
                                    st[:, 0:QC],
                                    kt_t[0:64, ksl],
                                    qt_t[0:64, qsl],
                                    start=True,
                                    stop=True,
                                )
                                nc.tensor.matmul(
                                    st[:, QC : 2 * QC],
                                    kt_t[64:128, ksl],
                                    qt_t[64:128, qsl],
                                    start=True,
                                    stop=True,
                                )
                            pt = pool.tile([P, 2 * QC], attn_dt, tag="pt0", bufs=xbufs)
                            nc.scalar.activation(
                                pt[:], st[:], mybir.ActivationFunctionType.Exp,
                                scale=SCALE,
                            )
                            if dbg and b == 0 and po == 0 and qc == 0 and kb == 0:
                                nc.sync.dma_start(d_pt0[:], pt[:].bitcast(F32))
                            first = kb == 0
                            last = kb == TT - 1
                            nc.tensor.matmul(
                                oa[:, :],
                                v_ext[:, kb, pb : pb + 128],
                                pt[:, 0:QC],
                                start=first,
                                stop=last,
                            )
                            nc.tensor.matmul(
                                ob[:, :],
                                v_ext[:, kb, pb + 32 : pb + 160],
                                pt[:, QC : 2 * QC],
                                start=first,
                                stop=last,
                            )
                            # splice next pair's QK-proj k-steps into the
                            # per-key-block exp hole: exp (1114ns) exceeds
                            # this slot's PE work (paired scores ~240ns +
                            # 2 PV matmuls ~450ns), leaving ~420ns for one
                            # filler item (2 matmuls) most slots.
                            if qk_work:
                                qk_work.pop(0)()
                        # epilogue: copy psum out early (frees oa/ob banks),
                        # then normalize by the ones-row sums
                        osb_a = pool.tile([P, QC], F32, tag="osb_a", bufs=xbufs)
                        osb_b = pool.tile([P, QC], F32, tag="osb_b", bufs=xbufs)
                        nc.vector.tensor_copy(osb_a[0:65, :], oa[0:65, :])
                        nc.vector.tensor_copy(osb_b[64:128, :], ob[64:128, :])
                        nc.vector.tensor_copy(osb_b[32:33, :], ob[32:33, :])
                        # denominators -> DRAM, reshaped to [128, 8] so the
                        # slow iterative DVE reciprocal uses all lanes, then
                        # broadcast back from DRAM (DMA partition-broadcast).
                        dden = dpool.tile([2, QC], F32, tag="dden", bufs=2)
                        nc.sync.dma_start(dden[0:1, :], osb_a[64:65, :])
                        nc.sync.dma_start(dden[1:2, :], osb_b[32:33, :])
                        den_sq = pool.tile([P, 8], F32, tag="den_sq", bufs=2)
                        nc.sync.dma_start(
                            den_sq[:],
                            dden[:].rearrange("a c -> (a c)").rearrange(
                                "(p f) -> p f", p=P
                            ),
                        )
                        rinv_sq = pool.tile([P, 8], F32, tag="rinv_sq", bufs=2)
                        nc.vector.reciprocal(rinv_sq[:], den_sq[:])
                        drin = dpool.tile([2, QC], F32, tag="drin", bufs=2)
                        nc.sync.dma_start(
                            drin[:].rearrange("a c -> (a c)").rearrange(
                                "(p f) -> p f", p=P
                            ),
                            rinv_sq[:],
                        )
                        rb = pool.tile([P, QC], F32, tag="rb", bufs=xbufs)
                        nc.sync.dma_start(
                            rb[0:64, :], drin[0:1, :].to_broadcast((64, QC))
                        )
                        nc.sync.dma_start(
                            rb[64:128, :], drin[1:2, :].to_broadcast((64, QC))
                        )
                        if dbg and b == 0 and po == 0 and qc == 0:
                            nc.sync.dma_start(d_osba[:], osb_a[:])
                            nc.sync.dma_start(d_osbb[:], osb_b[:])
                            nc.sync.dma_start(d_rb[:], rb[:])
                        nc.vector.tensor_mul(
                            ot[0:64, po, qsl], osb_a[0:64, :], rb[0:64, :]
                        )
                        nc.vector.tensor_mul(
                            ot[64:128, po, qsl], osb_b[64:128, :], rb[64:128, :]
                        )

                    while qk_work:
                        qk_work.pop(0)()
                    if po + 1 < NPAIR:
                        qt_t, kt_t = next_tiles

                if dbg and b == 0:
                    nc.sync.dma_start(d_ot[:], ot[:].bitcast(F32))

                # ---- Y = OT^T @ Wo + bo  (natural layout) ------------------
                for to in range(TT):
                    ystage = pool.tile([P, DIM], F32, tag="ystage", bufs=xbufs)
                    ypss = {
                        ch: ps.tile([P, QC], F32, tag="mm", bufs=2, name=f"yps{ch}")
                        for ch in (0, 1)
                    }
                    for k in range(KT):
                        for ch, cw in ((0, 512), (1, 256)):
                            nc.tensor.matmul(
                                ypss[ch][:, :cw],
                                ot[:, k, to * P : (to + 1) * P],
                                wo_sb[:, k, ch * 512 : ch * 512 + cw],
                                start=(k == 0),
                                stop=(k == KT - 1),
                            )
                    for ch, cw in ((0, 512), (1, 256)):
                        nc.vector.scalar_tensor_tensor(
                            out=ystage[:, ch * 512 : ch * 512 + cw],
                            in0=ypss[ch][:, :cw],
                            scalar=1.0,
                            in1=bo_b[:, ch * 512 : ch * 512 + cw],
                            op0=mybir.AluOpType.mult,
                            op1=mybir.AluOpType.add,
                        )
                    nc.sync.dma_start(
                        out[b, to * P : (to + 1) * P, :], ystage[:]
                    )

    nc.finalize()
    return nc


def _run(inputs: dict, mm_dtype=None, attn_bf16=True, trace: bool = False, dbg: bool = False):
    # bf16 projections beat f32r: the PE moving port is 2 B/lane/cycle, so
    # f32r streams at 2 cycles/col while bf16 streams at 1 (measured
    # ~434us bf16 vs ~500us f32r same-session; rel err 3.8e-3 vs 1.7e-3,
    # both far under the 2e-2 budget).
    if mm_dtype is None:
        mm_dtype = mybir.dt.bfloat16
    key = (str(mm_dtype), attn_bf16, dbg)
    if key not in _cache:
        # warm=True streams dummy matmuls under the load prologue so a
        # cold (down-clocked) device ramps the PE p-state before real work
        _cache[key] = build(mm_dtype, attn_bf16=attn_bf16, dbg=dbg, warm=True)
    nc = _cache[key]
    return _run_nc(nc, inputs, trace)


def _run_nc(nc, inputs: dict, trace: bool = False):

    x = np.ascontiguousarray(inputs["inputs"], dtype=np.float32)
    shared = {
        k: np.ascontiguousarray(inputs[k], dtype=np.float32)
        for k in ("Wq", "bq", "Wk", "bk", "Wv", "bv", "Wo", "bo")
    }
    in_maps = [
        {"inputs": x[c * BL : (c + 1) * BL], **shared} for c in range(NCORES)
    ]
    res = run_bass_kernel_spmd(nc, in_maps, list(range(NCORES)), trace=trace)
    full = np.concatenate([res.results[c]["out"] for c in range(NCORES)], axis=0)
    return full, res


def kernel(**inputs) -> np.ndarray:
    out, _ = _run(inputs)
    return out



# revision 18
# speedup vs baseline: 1.1702x; 1.1702x over previous
"""Multi-head attention kernel for Trainium2, data-parallel over batch on 8 cores.

Problem: B=16, N=1024, DIM=768, H=12 heads, head_dim=64, fp32.
  q = x@Wq+bq; k = x@Wk+bk; v = x@Wv+bv   (per-head split)
  out = softmax(q k^T / sqrt(DIM)) v      (per head), concat, @Wo + bo

Sharding: batch-parallel. Each core gets 2 batches and all weights; no
collectives. Output gathered by concat.

Per-core layout strategy (per batch of 1024 tokens):
  - XT = x^T  [768 feat, 1024 tok] via PE transposes (fp32 DMA transpose
    unsupported).
  - QT/KT = (x@W + b)^T [768, 1024]: matmul(lhsT=W, rhs=XT). Head h lives on
    partition rows (h%2)*64..: pair p = m-tile p.
  - V natural [1024 tok, 768] via matmul(lhsT=XT, rhs=Wv), stored per-pair
    padded: [Vh0(64) | ones(1) | pad(31) | Vh1(64)] = 160 cols. The shared
    ones column makes PV emit softmax denominators at 32-aligned psum rows:
      h0: lhsT cols [0:128]  -> psum rows 0-63 = O_h0^T, row 64 = denom_h0
      h1: lhsT cols [32:160] -> psum row 32 = denom_h1, rows 64-127 = O_h1^T
         (remaining rows garbage, never read)
  - S^T[key, q] = matmul(lhsT=KT head rows, rhs=QT head rows), contraction 64,
    two heads row-packed in the PE array (partitions 0-63 / 64-127).
  - P^T = exp(SCALE * S^T) on ACT (no max subtraction needed: |SCALE*S| < ~2),
    [128, 1024] ops (2 key-blocks per op) to amortize ACT overhead.
  - O^T normalized by broadcast reciprocal rows, written to OT [768, 1024].
  - Y = matmul(lhsT=OT, rhs=Wo) + bo -> natural [tok, 768], DMA out.

All matmuls (projections and attention) run in bf16 with fp32 psum
accumulation: the PE moving-operand port is 2 B/lane/cycle on TRN2, so bf16
streams 1 col/cycle while f32r needs 2 — bf16 projections measure ~65us
faster end-to-end than f32r (434us vs 500us same-session), at rel err
~3.8e-3 of the output absmax (budget 2e-2). x is cast to bf16 in DRAM and
transposed via the XBAR DMA-transpose, so the PE does no transpose work.
"""

import sys
import types

sys.path.insert(0, "/opt/trn_rl_repo")

import numpy as np

# Register the axon NTFF profile hook if the image's antenv lacks it (needed
# only when run with trace=True; harmless otherwise).
import antenv  # noqa: F401

if "antenv.axon_hooks" not in sys.modules:
    _hooks_mod = types.ModuleType("antenv.axon_hooks")
    _hooks_mod._hook = None

    def _set_hook(h):
        _hooks_mod._hook = h

    def _get_hook():
        return _hooks_mod._hook

    _hooks_mod.set_axon_ntff_profile_hook = _set_hook
    _hooks_mod.get_axon_ntff_profile_hook = _get_hook
    sys.modules["antenv.axon_hooks"] = _hooks_mod
    try:
        from trn_agent_boot.trn_boot import _ntff_profile_via_ctypes

        _set_hook(_ntff_profile_via_ctypes("/opt/axon/libaxon_pjrt.so"))
    except Exception:
        pass

import concourse.bass_utils as bass_utils

bass_utils.upload_artifacts = lambda tmpdir: f"local:{tmpdir}"  # no bucket creds

import concourse.bacc as bacc
import concourse.mybir as mybir
import concourse.tile as tile
from concourse.bass_utils import run_bass_kernel_spmd
from concourse.masks import make_identity

P = 128
DIM = 768
N_HEADS = 12
HD = 64
N = 1024
B = 16
NCORES = 8
BL = B // NCORES  # batches per core = 2
SCALE = 1.0 / float(np.sqrt(DIM))

KT = DIM // P      # 6 k-tiles of the 768 contraction
TT = N // P        # 8 token tiles per batch
NPAIR = N_HEADS // 2  # 6 head pairs
QC = 512           # query chunk (psum bank, fp32)
PAIRW = 160        # pair block in V_ext: [Vh0(64)|ones(1)|pad(31)|Vh1(64)]

F32 = mybir.dt.float32

_cache = {}


def build(mm_dtype, attn_bf16=True, dbg=False, warm=False):
    nc = bacc.Bacc("TRN2", target_bir_lowering=False, debug=False)

    x = nc.dram_tensor("inputs", [BL, N, DIM], F32, kind="ExternalInput")
    wq = nc.dram_tensor("Wq", [DIM, DIM], F32, kind="ExternalInput")
    bq = nc.dram_tensor("bq", [DIM], F32, kind="ExternalInput")
    wk = nc.dram_tensor("Wk", [DIM, DIM], F32, kind="ExternalInput")
    bk = nc.dram_tensor("bk", [DIM], F32, kind="ExternalInput")
    wv = nc.dram_tensor("Wv", [DIM, DIM], F32, kind="ExternalInput")
    bv = nc.dram_tensor("bv", [DIM], F32, kind="ExternalInput")
    wo = nc.dram_tensor("Wo", [DIM, DIM], F32, kind="ExternalInput")
    bo = nc.dram_tensor("bo", [DIM], F32, kind="ExternalInput")
    out = nc.dram_tensor("out", [BL, N, DIM], F32, kind="ExternalOutput")
    if dbg:
        d_xt = nc.dram_tensor("d_xt", [P, KT, N], F32, kind="ExternalOutput")
        d_vext = nc.dram_tensor("d_vext", [P, TT, NPAIR * PAIRW], F32, kind="ExternalOutput")
        d_qt = nc.dram_tensor("d_qt", [P, N], F32, kind="ExternalOutput")
        d_kt = nc.dram_tensor("d_kt", [P, N], F32, kind="ExternalOutput")
        d_pt0 = nc.dram_tensor("d_pt0", [P, 2 * QC], F32, kind="ExternalOutput")
        d_pt1 = nc.dram_tensor("d_pt1", [P, 2 * QC], F32, kind="ExternalOutput")
        d_osba = nc.dram_tensor("d_osba", [P, QC], F32, kind="ExternalOutput")
        d_osbb = nc.dram_tensor("d_osbb", [P, QC], F32, kind="ExternalOutput")
        d_rb = nc.dram_tensor("d_rb", [P, QC], F32, kind="ExternalOutput")
        d_ot = nc.dram_tensor("d_ot", [P, KT, N], F32, kind="ExternalOutput")

    wq_r = wq.rearrange("(ko ki) m -> ki ko m", ki=P)
    wk_r = wk.rearrange("(ko ki) m -> ki ko m", ki=P)
    wv_r = wv.rearrange("(ko ki) m -> ki ko m", ki=P)
    wo_r = wo.rearrange("(ko ki) m -> ki ko m", ki=P)
    bq_r = bq.rearrange("(ko ki) -> ki ko", ki=P)
    bk_r = bk.rearrange("(ko ki) -> ki ko", ki=P)

    # weights DMA: gpsimd can cast f32 -> f32r/bf16 in flight
    wdma = nc.sync.dma_start if mm_dtype == F32 else nc.gpsimd.dma_start
    attn_dt = mybir.dt.bfloat16 if attn_bf16 else mm_dtype
    xbufs = 3 if mm_dtype == mybir.dt.bfloat16 else 2

    with tile.TileContext(nc) as tc:
        with (
            tc.tile_pool(name="const", bufs=1) as cpool,
            tc.tile_pool(name="work", bufs=1) as pool,
            tc.tile_pool(name="dram", bufs=1, space="DRAM") as dpool,
            tc.tile_pool(name="ps", bufs=1, space="PSUM") as ps,
        ):
            ident = cpool.tile([P, P], F32)
            make_identity(nc, ident)

            if warm:
                # dummy matmuls to hold the DVFS p-state up while the x cast
                # + XBAR transpose + weight-load prologue runs (PE otherwise
                # idles at kernel start and the clock halves). The pipelined
                # per-token-block transposes get real work going ~12us in, so
                # 32 warm matmuls suffice to bridge.
                wrm = cpool.tile([P, QC], mybir.dt.bfloat16)
                nc.vector.memset(wrm[:], 0.0)
                for _ in range(16):
                    wps = ps.tile([P, QC], F32, tag="mm", bufs=2, name="wps")
                    nc.tensor.matmul(
                        wps[:], wrm[:, 0:P], wrm[:], start=True, stop=True
                    )

            bf16_x = mm_dtype == mybir.dt.bfloat16

            # resident weights (full); order by first use. With x loaded on
            # the sync queue (below), the gpsimd cast queue carries ONLY the
            # weights, so wv lands ~9us in and wq/wk by ~23us.
            wq_sb = cpool.tile([P, KT, DIM], mm_dtype)
            wk_sb = cpool.tile([P, KT, DIM], mm_dtype)
            wv_sb = cpool.tile([P, KT, DIM], mm_dtype)
            wo_sb = cpool.tile([P, KT, DIM], mm_dtype)
            for k in range(KT):
                wdma(wv_sb[:, k], wv_r[:, k])
            for k in range(KT):
                wdma(wq_sb[:, k], wq_r[:, k])
                wdma(wk_sb[:, k], wk_r[:, k])
            for k in range(KT):
                wdma(wo_sb[:, k], wo_r[:, k])

            bq_sb = cpool.tile([P, KT], F32)
            bk_sb = cpool.tile([P, KT], F32)
            bv_b = cpool.tile([P, DIM], F32)
            bo_b = cpool.tile([P, DIM], F32)

            # V_ext: [tok_inner, tok_outer, pair blocks of PAIRW cols]
            # cols p*PAIRW + [0:64] = V head 2p, +64 = ones, +[96:160] = V 2p+1
            # pad cols stay uninitialized: they only produce garbage psum rows
            # that are never read. Ones col via DVE cast-copy (f32r producer).
            v_ext = cpool.tile([P, TT, NPAIR * PAIRW], attn_dt)
            ones_src = cpool.tile([P, TT * NPAIR], F32)
            nc.vector.memset(ones_src[:], 1.0)
            ones_cols = v_ext[:].rearrange("p t (np w) -> p t np w", w=PAIRW)[
                :, :, :, 64:65
            ]
            nc.vector.tensor_copy(
                ones_cols,
                ones_src[:].rearrange("p (t np) -> p t np", np=NPAIR)[:, :, :, None],
            )

            # XT via PE transposes: x token blocks stream in as plain f32 on
            # the otherwise-idle sync queue (no cast round-trip through DRAM,
            # no slow small-block XBAR transposes); the PE transposes each
            # 128x128 chunk (f32 transpose = 2 cycles/row, ~107ns/chunk, and
            # doubles as DVFS warm-up work) and the DVE copies psum -> xt in
            # bf16. Both batches run here: b1's transposes fill the wv-paced
            # bubbles of b0's V projection, and issuing its x loads late
            # would park them behind b0's attention-epilogue DMAs (FIFO).
            xts = []

            def _transpose_x(b):
                # psum -> xt copies ride the Scalar (ACT) engine, idle until
                # the first exp ~30us in: on the DVE they contend with the
                # V-proj bias-scatter ops and stall the PE ~3us per token
                # block (gpsimd cannot read PSUM — NEFF compile rejects it).
                for to in range(TT):
                    tsl = slice(to * P, (to + 1) * P)
                    xstage = pool.tile([P, DIM], F32, tag="xstage", bufs=3)
                    nc.sync.dma_start(xstage[:], x[b, tsl, :])
                    tp0 = ps.tile([P, QC], F32, tag="mm", bufs=2, name="tp0")
                    tp1 = ps.tile([P, QC], F32, tag="mm", bufs=2, name="tp1")
                    for fo in range(KT):
                        dst = tp0 if fo < 4 else tp1
                        nc.tensor.transpose(
                            dst[:, (fo % 4) * P : (fo % 4 + 1) * P],
                            xstage[:, fo * P : (fo + 1) * P],
                            ident,
                        )
                    nc.scalar.activation(
                        xts[b][:, 0:4, tsl],
                        tp0[:].rearrange("p (f t) -> p f t", f=4),
                        mybir.ActivationFunctionType.Copy,
                    )
                    nc.scalar.activation(
                        xts[b][:, 4:6, tsl],
                        tp1[:, 0:256].rearrange("p (f t) -> p f t", f=2),
                        mybir.ActivationFunctionType.Copy,
                    )

            if bf16_x:
                for b in range(BL):
                    xt_b = pool.tile(
                        [P, KT, N], mm_dtype, tag="xt_ot", bufs=xbufs, name=f"xt{b}"
                    )
                    xts.append(xt_b)
                _transpose_x(0)
                # b1's transposes are emitted later (inside b0's V-proj
                # phase) so they fill that phase's DMA-paced PE bubbles
                # instead of competing with b0's transposes up front.

            for b in range(BL):
                # ---- XT = x[b]^T ------------------------------------------
                if b == 0:
                    nc.scalar.dma_start(bv_b[:], bv[None, :].to_broadcast((P, DIM)))
                    nc.scalar.dma_start(bq_sb[:], bq_r)
                    nc.scalar.dma_start(bk_sb[:], bk_r)
                    nc.scalar.dma_start(bo_b[:], bo[None, :].to_broadcast((P, DIM)))
                if bf16_x:
                    xt = xts[b]
                else:
                    xt = pool.tile(
                        [P, KT, N], mm_dtype, tag="xt_ot", bufs=xbufs, name="xt"
                    )
                    for to in range(TT):
                        xstage = pool.tile([P, DIM], F32, tag="xstage", bufs=2)
                        nc.sync.dma_start(xstage[:], x[b, to * P : (to + 1) * P, :])
                        for fo in range(KT):
                            tps = ps.tile([P, QC], F32, tag="mm", bufs=2, name="tps")
                            nc.tensor.transpose(
                                tps[:, :P], xstage[:, fo * P : (fo + 1) * P], ident
                            )
                            nc.vector.tensor_copy(
                                xt[:, fo, to * P : (to + 1) * P], tps[:, :P]
                            )

                # ---- V natural + ones layout ------------------------------
                for to in range(TT):
                    vpss = {
                        ch: ps.tile([P, QC], F32, tag="mm", bufs=2, name=f"vps{ch}")
                        for ch in (0, 1)
                    }
                    for k in range(KT):
                        for ch, cw in ((0, 512), (1, 256)):
                            nc.tensor.matmul(
                                vpss[ch][:, :cw],
                                xt[:, k, to * P : (to + 1) * P],
                                wv_sb[:, k, ch * 512 : ch * 512 + cw],
                                start=(k == 0),
                                stop=(k == KT - 1),
                            )
                    for ch, cw in ((0, 512), (1, 256)):
                        vps = vpss[ch]
                        # scatter heads into pair-padded blocks (+bias)
                        npr = cw // (2 * HD)  # pairs in this chunk (4 then 2)
                        pr0 = ch * 4          # first pair in this chunk
                        for par in (0, 1):    # even/odd head of each pair
                            src = vps[:, :cw].rearrange(
                                "p (np two w) -> p np two w", two=2, w=HD
                            )[:, :, par, :]
                            bsrc = bv_b[:, ch * 512 : ch * 512 + cw].rearrange(
                                "p (np two w) -> p np two w", two=2, w=HD
                            )[:, :, par, :]
                            off = 96 if par else 0
                            dst = v_ext[:, to, :].rearrange(
                                "p (np w) -> p np w", w=PAIRW
                            )[:, pr0 : pr0 + npr, off : off + HD]
                            nc.vector.scalar_tensor_tensor(
                                out=dst,
                                in0=src,
                                scalar=1.0,
                                in1=bsrc,
                                op0=mybir.AluOpType.mult,
                                op1=mybir.AluOpType.add,
                            )

                if dbg and b == 0:
                    nc.sync.dma_start(d_xt[:], xt[:].bitcast(F32))
                    nc.sync.dma_start(d_vext[:], v_ext[:].bitcast(F32))

                if bf16_x and b == 0:
                    # b1's x transposes: emitted here so they slot into the
                    # wv/wq-DMA-paced bubbles of b0's V-proj + QK0 phase.
                    _transpose_x(1)

                # ---- OT buffer for this batch -----------------------------
                ot = pool.tile([P, KT, N], mm_dtype, tag="xt_ot", bufs=xbufs, name="ot")

                # ---- per head-pair: QT/KT proj then attention -------------
                def make_qk_work(npo, nqt, nkt):
                    """QK projection for pair npo as a list of closures (one
                    k-step of one dst each; the last also adds the biases) so
                    it can interleave into the PREVIOUS pair's attention — the
                    PE queue is in-order, so the exp-wait stalls there would
                    otherwise leave the PE idle while this work sits queued
                    behind them."""
                    work = []
                    for dst_t, w_t, bias in ((nqt, wq_sb, bq_sb), (nkt, wk_sb, bk_sb)):
                        holder = {}

                        def chunk(k, dst_t=dst_t, w_t=w_t, bias=bias, holder=holder):
                            if k == 0:
                                holder["pp"] = [
                                    ps.tile([P, QC], F32, tag="mm", bufs=2, name=f"pps{qs}")
                                    for qs in range(N // QC)
                                ]
                            for qs in range(N // QC):
                                nc.tensor.matmul(
                                    holder["pp"][qs][:],
                                    w_t[:, k, npo * P : (npo + 1) * P],
                                    xt[:, k, qs * QC : (qs + 1) * QC],
                                    start=(k == 0),
                                    stop=(k == KT - 1),
                                )
                            if k == KT - 1:
                                for qs in range(N // QC):
                                    nc.vector.tensor_scalar_add(
                                        dst_t[:, qs * QC : (qs + 1) * QC],
                                        holder["pp"][qs][:],
                                        bias[:, npo : npo + 1],
                                    )

                        for k in range(KT):
                            work.append(lambda k=k, chunk=chunk: chunk(k))
                    return work

                # pair 0's QK projection runs inline; later pairs' are
                # interleaved into the preceding pair's attention loop
                qt_t = pool.tile([P, N], attn_dt, tag="qt", bufs=xbufs)
                kt_t = pool.tile([P, N], attn_dt, tag="kt", bufs=xbufs)
                for w0 in make_qk_work(0, qt_t, kt_t):
                    w0()

                for po in range(NPAIR):
                    qk_work = []
                    if po + 1 < NPAIR:
                        nqt = pool.tile([P, N], attn_dt, tag="qt", bufs=xbufs)
                        nkt = pool.tile([P, N], attn_dt, tag="kt", bufs=xbufs)
                        qk_work = make_qk_work(po + 1, nqt, nkt)
                        next_tiles = (nqt, nkt)

                    if dbg and b == 0 and po == 0:
                        nc.sync.dma_start(d_qt[:], qt_t[:].bitcast(F32))
                        nc.sync.dma_start(d_kt[:], kt_t[:].bitcast(F32))

                    pb = po * PAIRW
                    for qc in range(N // QC):
                        qsl = slice(qc * QC, (qc + 1) * QC)
                        oa = ps.tile([P, QC], F32, tag="oa", bufs=1, name="oa")
                        ob = ps.tile([P, QC], F32, tag="ob", bufs=1, name="ob")
                        for kb in range(TT):
                            ksl = slice(kb * P, (kb + 1) * P)
                            # ONE st tile per key block holding BOTH heads
                            # ([h0 512 | h1 512]) so a single exp frees both
                            # banks at once: the two score matmuls then become
                            # ready together and issue back-to-back, streaming
                            # CONCURRENTLY on the two 64-row PE tile halves
                            # (h0 rows 0-63, h1 rows 64-127) — 2 cols/cycle.
                            # With separate st0/st1 + two exps, h1's bank
                            # frees ~1.1us after h0's and the scheduler runs
                            # the h0 matmuls solo, breaking the pairing.
                            st = ps.tile([P, 2 * QC], F32, tag="st", bufs=2, name="st")
                            # high_priority keeps the pair adjacent in the
                            # final schedule: filler k-steps become ready
                            # one-at-a-time (chained psum accumulation) and
                            # would otherwise slip between h0 and h64 with
                            # their lower (earlier) priorities.
                            with tc.high_priority(offset=256):
                                nc.tensor.matmul(
                                    st[:, 0:QC],
                                    kt_t[0:64, ksl],
                                    qt_t[0:64, qsl],
                                    start=True,
                                    stop=True,
                                )
                                nc.tensor.matmul(
                                    st[:, QC : 2 * QC],
                                    kt_t[64:128, ksl],
                                    qt_t[64:128, qsl],
                                    start=True,
                                    stop=True,
                                )
                            pt = pool.tile([P, 2 * QC], attn_dt, tag="pt0", bufs=xbufs)
                            nc.scalar.activation(
                                pt[:], st[:], mybir.ActivationFunctionType.Exp,
                                scale=SCALE,
                            )
                            if dbg and b == 0 and po == 0 and qc == 0 and kb == 0:
                                nc.sync.dma_start(d_pt0[:], pt[:].bitcast(F32))
                            first = kb == 0
                            last = kb == TT - 1
                            nc.tensor.matmul(
                                oa[:, :],
                                v_ext[:, kb, pb : pb + 128],
                                pt[:, 0:QC],
                                start=first,
                                stop=last,
                            )
                            nc.tensor.matmul(
                                ob[:, :],
                                v_ext[:, kb, pb + 32 : pb + 160],
                                pt[:, QC : 2 * QC],
                                start=first,
                                stop=last,
                            )
                            # splice next pair's QK-proj k-steps into the
                            # per-key-block exp hole: exp (1114ns) exceeds
                            # this slot's PE work (paired scores ~240ns +
                            # 2 PV matmuls ~450ns), leaving ~420ns for one
                            # filler item (2 matmuls) most slots.
                            if qk_work:
                                qk_work.pop(0)()
                        # epilogue: copy psum out early (frees oa/ob banks),
                        # then normalize by the ones-row sums
                        osb_a = pool.tile([P, QC], F32, tag="osb_a", bufs=xbufs)
                        osb_b = pool.tile([P, QC], F32, tag="osb_b", bufs=xbufs)
                        nc.vector.tensor_copy(osb_a[0:65, :], oa[0:65, :])
                        nc.vector.tensor_copy(osb_b[64:128, :], ob[64:128, :])
                        nc.vector.tensor_copy(osb_b[32:33, :], ob[32:33, :])
                        # denominators -> DRAM, reshaped to [128, 8] so the
                        # slow iterative DVE reciprocal uses all lanes, then
                        # broadcast back from DRAM (DMA partition-broadcast).
                        dden = dpool.tile([2, QC], F32, tag="dden", bufs=2)
                        nc.sync.dma_start(dden[0:1, :], osb_a[64:65, :])
                        nc.sync.dma_start(dden[1:2, :], osb_b[32:33, :])
                        den_sq = pool.tile([P, 8], F32, tag="den_sq", bufs=2)
                        nc.sync.dma_start(
                            den_sq[:],
                            dden[:].rearrange("a c -> (a c)").rearrange(
                                "(p f) -> p f", p=P
                            ),
                        )
                        rinv_sq = pool.tile([P, 8], F32, tag="rinv_sq", bufs=2)
                        nc.vector.reciprocal(rinv_sq[:], den_sq[:])
                        drin = dpool.tile([2, QC], F32, tag="drin", bufs=2)
                        nc.sync.dma_start(
                            drin[:].rearrange("a c -> (a c)").rearrange(
                                "(p f) -> p f", p=P
                            ),
                            rinv_sq[:],
                        )
                        rb = pool.tile([P, QC], F32, tag="rb", bufs=xbufs)
                        nc.sync.dma_start(
                            rb[0:64, :], drin[0:1, :].to_broadcast((64, QC))
                        )
                        nc.sync.dma_start(
                            rb[64:128, :], drin[1:2, :].to_broadcast((64, QC))
                        )
                        if dbg and b == 0 and po == 0 and qc == 0:
                            nc.sync.dma_start(d_osba[:], osb_a[:])
                            nc.sync.dma_start(d_osbb[:], osb_b[:])
                            nc.sync.dma_start(d_rb[:], rb[:])
                        nc.vector.tensor_mul(
                            ot[0:64, po, qsl], osb_a[0:64, :], rb[0:64, :]
                        )
                        nc.vector.tensor_mul(
                            ot[64:128, po, qsl], osb_b[64:128, :], rb[64:128, :]
                        )

                    while qk_work:
                        qk_work.pop(0)()
                    if po + 1 < NPAIR:
                        qt_t, kt_t = next_tiles

                if dbg and b == 0:
                    nc.sync.dma_start(d_ot[:], ot[:].bitcast(F32))

                # ---- Y = OT^T @ Wo + bo  (natural layout) ------------------
                for to in range(TT):
                    ystage = pool.tile([P, DIM], F32, tag="ystage", bufs=xbufs)
                    ypss = {
                        ch: ps.tile([P, QC], F32, tag="mm", bufs=2, name=f"yps{ch}")
                        for ch in (0, 1)
                    }
                    for k in range(KT):
                        for ch, cw in ((0, 512), (1, 256)):
                            nc.tensor.matmul(
                                ypss[ch][:, :cw],
                                ot[:, k, to * P : (to + 1) * P],
                                wo_sb[:, k, ch * 512 : ch * 512 + cw],
                                start=(k == 0),
                                stop=(k == KT - 1),
                            )
                    for ch, cw in ((0, 512), (1, 256)):
                        nc.vector.scalar_tensor_tensor(
                            out=ystage[:, ch * 512 : ch * 512 + cw],
                            in0=ypss[ch][:, :cw],
                            scalar=1.0,
                            in1=bo_b[:, ch * 512 : ch * 512 + cw],
                            op0=mybir.AluOpType.mult,
                            op1=mybir.AluOpType.add,
                        )
                    nc.sync.dma_start(
                        out[b, to * P : (to + 1) * P, :], ystage[:]
                    )

    nc.finalize()
    return nc


def _run(inputs: dict, mm_dtype=None, attn_bf16=True, trace: bool = False, dbg: bool = False):
    # bf16 projections beat f32r: the PE moving port is 2 B/lane/cycle, so
    # f32r streams at 2 cycles/col while bf16 streams at 1 (measured
    # ~434us bf16 vs ~500us f32r same-session; rel err 3.8e-3 vs 1.7e-3,
    # both far under the 2e-2 budget).
    if mm_dtype is None:
        mm_dtype = mybir.dt.bfloat16
    key = (str(mm_dtype), attn_bf16, dbg)
    if key not in _cache:
        # warm=True streams dummy matmuls under the load prologue so a
        # cold (down-clocked) device ramps the PE p-state before real work
        _cache[key] = build(mm_dtype, attn_bf16=attn_bf16, dbg=dbg, warm=True)
    nc = _cache[key]
    return _run_nc(nc, inputs, trace)


def _run_nc(nc, inputs: dict, trace: bool = False):

    x = np.ascontiguousarray(inputs["inputs"], dtype=np.float32)
    shared = {
        k: np.ascontiguousarray(inputs[k], dtype=np.float32)
        for k in ("Wq", "bq", "Wk", "bk", "Wv", "bv", "Wo", "bo")
    }
    in_maps = [
        {"inputs": x[c * BL : (c + 1) * BL], **shared} for c in range(NCORES)
    ]
    res = run_bass_kernel_spmd(nc, in_maps, list(range(NCORES)), trace=trace)
    full = np.concatenate([res.results[c]["out"] for c in range(NCORES)], axis=0)
    return full, res


def kernel(**inputs) -> np.ndarray:
    out, _ = _run(inputs)
    return out

